# revision 6
# baseline (speedup 1.0000x reference)
"""Single-head causal attention on 8 Trainium2 NeuronCores.

Problem: x:[8,2048,1024], Wq/Wk/Wv:[64,1024], bq/bk/bv:[64]
  q,k,v = x@W*.T + b*;  out = softmax(causal(q@k.T)/sqrt(64)) @ v

Sharding: batch dim (8) across the 8 cores — fully data-parallel, no
collectives. Each core computes one batch's attention head.

Per-core device kernel (all matmuls fp32r = tf32, full-rate at N=512):
  - host supplies x transposed (xT [E,S]) so E (the contraction dim of the
    projections) lands on SBUF partitions.
  - projections: psum_qk[128,512] accumulates [Wq|Wk].T packed (M=128),
    psum_v[64,512] accumulates Wv.T, over 8 e-tiles of 128.
  - q is pre-scaled by 1/sqrt(64) (scale folded into the PSUM->SBUF copy,
    bias folded there too), so scores come out pre-scaled.
  - scores kept transposed: sT[k,q] = kT.T @ qT per (k-tile 128, q-chunk 512).
    Causally-dead tiles are skipped outright; diagonal tiles are masked by
    multiplying exp(s) with a 0/1 ramp mask (slices of one [128,896] ramp).
  - softmax without max-subtraction (scores/8 ~ N(0,1); max < ~6, exp safe
    in fp32) -> denominator = column sum of exp(sT), obtained for free as
    row 64 of the attention*V matmul by appending a ones-row to V.
  - V needs k on partitions for the AV matmul: vT tiles are transposed
    128-column-wise on the PE (identity matmul).
  - normalize on device: reciprocal of the denominator row, broadcast
    across the 64 head partitions with a K=1 matmul, multiply.
  - output written h-major ([64, 2048]); host transposes back.
"""

import numpy as np

import concourse.bacc as bacc
import concourse.mybir as mybir
import concourse.tile as tile
from concourse import bass2jax

B, S, E, H = 8, 2048, 1024, 64
NCORES = 8
PB = 128  # partition block / k-tile size
QB = 512  # q-chunk (matmul moving free dim)
ET = E // PB  # e-tiles per contraction
QC = S // QB  # q-chunks
KT = S // PB  # k-tiles
DIAG = QB // PB  # diagonal k-tiles per q-chunk

F32 = mybir.dt.float32
F32R = mybir.dt.float32r
AF = mybir.ActivationFunctionType
MUL = mybir.AluOpType.mult

_CACHE: dict = {}


def _build_nc():
    nc = bacc.Bacc("TRN2", target_bir_lowering=False, debug=False)
    xT = nc.dram_tensor("xT", [E, S], F32R, kind="ExternalInput").ap()
    wqk = nc.dram_tensor("wqk", [E, 2 * H], F32R, kind="ExternalInput").ap()
    wv = nc.dram_tensor("wv", [E, H], F32R, kind="ExternalInput").ap()
    sb_in = nc.dram_tensor("sb", [PB, 2], F32, kind="ExternalInput").ap()
    bv_in = nc.dram_tensor("bv", [H, 1], F32, kind="ExternalInput").ap()
    mask_in = nc.dram_tensor("mask", [PB, QB + 384], F32R, kind="ExternalInput").ap()
    id_in = nc.dram_tensor("ident", [H, H], F32, kind="ExternalInput").ap()
    ones_in = nc.dram_tensor("ones", [PB, H], F32R, kind="ExternalInput").ap()
    out = nc.dram_tensor("out", [H, S], F32, kind="ExternalOutput").ap()

    with tile.TileContext(nc) as tc:
        with (
            tc.tile_pool(name="const", bufs=1) as constp,
            tc.tile_pool(name="xs", bufs=4) as xpool,
            tc.tile_pool(name="qkv", bufs=1) as qkvp,
            tc.tile_pool(name="wt", bufs=3) as wtp,
            tc.tile_pool(name="fin", bufs=2) as finp,
            tc.tile_pool(name="pqk", bufs=2, space="PSUM") as pqk,
            tc.tile_pool(name="pv", bufs=2, space="PSUM") as pvp,
            tc.tile_pool(name="ps", bufs=2, space="PSUM") as psp,
            tc.tile_pool(name="pav", bufs=2, space="PSUM") as pavp,
        ):
            wqk_sb = constp.tile([PB, ET, 2 * H], F32R)
            nc.sync.dma_start(wqk_sb[:], wqk.rearrange("(t p) m -> p t m", p=PB))
            wv_sb = constp.tile([PB, ET, H], F32R)
            nc.sync.dma_start(wv_sb[:], wv.rearrange("(t p) m -> p t m", p=PB))
            sb_sb = constp.tile([PB, 2], F32)
            nc.sync.dma_start(sb_sb[:], sb_in[:])
            bv_sb = constp.tile([H, 1], F32)
            nc.sync.dma_start(bv_sb[:], bv_in[:])
            mask_sb = constp.tile([PB, QB + 384], F32R)
            nc.sync.dma_start(mask_sb[:], mask_in[:])
            id_sb = constp.tile([H, H], F32)
            nc.sync.dma_start(id_sb[:], id_in[:])
            ones_sb = constp.tile([PB, H], F32R)
            nc.sync.dma_start(ones_sb[:], ones_in[:])

            qkT = qkvp.tile([PB, S], F32R)  # rows 0:64 = q/8, 64:128 = k
            kT = qkvp.tile([H, S], F32R)  # k re-based to partitions 0:64
            vT = qkvp.tile([H, S], F32)  # v h-major (bias applied)
            vsb = qkvp.tile([PB, KT, H + 1], F32R)  # v k-major + ones col
            for m in range(KT):
                nc.vector.tensor_copy(vsb[:, m, H : H + 1], ones_sb[:, 0:1])

            def proj(c):
                qs = slice(c * QB, (c + 1) * QB)
                p_qk = pqk.tile([PB, QB], F32, tag="pqk")
                p_v = pvp.tile([H, QB], F32, tag="pv")
                for e in range(ET):
                    xt = xpool.tile([PB, QB], F32R, tag="xt")
                    nc.sync.dma_start(xt[:], xT[e * PB : (e + 1) * PB, qs])
                    nc.tensor.matmul(
                        p_qk[:], wqk_sb[:, e, :], xt[:], start=(e == 0), stop=(e == ET - 1)
                    )
                    nc.tensor.matmul(
                        p_v[:], wv_sb[:, e, :], xt[:], start=(e == 0), stop=(e == ET - 1)
                    )
                nc.scalar.activation(
                    qkT[:, qs], p_qk[:], AF.Identity, bias=sb_sb[:, 1:2], scale=sb_sb[:, 0:1]
                )
                nc.scalar.activation(vT[:, qs], p_v[:], AF.Identity, bias=bv_sb[:])
                nc.sync.dma_start(kT[:, qs], qkT[H:PB, qs])
                for t in range(DIAG):
                    m = DIAG * c + t
                    p_vt = pvp.tile([PB, H], F32, tag="pv")
                    nc.tensor.transpose(p_vt[:], vT[:, m * PB : (m + 1) * PB], id_sb[:])
                    nc.vector.tensor_copy(vsb[:, m, 0:H], p_vt[:])

            def attn(c):
                qs = slice(c * QB, (c + 1) * QB)
                nkt = DIAG * c + DIAG
                p_av = pavp.tile([H + 1, QB], F32, tag="pav")
                for m in range(nkt):
                    p_s = psp.tile([PB, QB], F32, tag="ps")
                    nc.tensor.matmul(
                        p_s[:],
                        kT[:, m * PB : (m + 1) * PB],
                        qkT[0:H, qs],
                        start=True,
                        stop=True,
                    )
                    w = wtp.tile([PB, QB], F32R, tag="w")
                    nc.scalar.activation(w[:], p_s[:], AF.Exp)
                    i = m - DIAG * c
                    if i >= 0:
                        nc.vector.tensor_tensor(
                            w[:], w[:], mask_sb[:, 384 - PB * i : 384 - PB * i + QB], MUL
                        )
                    nc.tensor.matmul(
                        p_av[:], vsb[:, m, :], w[:], start=(m == 0), stop=(m == nkt - 1)
                    )
                # normalize: out[h, q] = av[h, q] / av[64, q]
                dn = wtp.tile([PB, QB], F32R, tag="w")
                with nc.allow_low_precision("fp32r denominators feed an fp32r matmul"):
                    nc.vector.reciprocal(dn[H : H + 1, :], p_av[H : H + 1, :])
                p_rep = psp.tile([PB, QB], F32, tag="ps")
                nc.tensor.matmul(
                    p_rep[0:H, :],
                    ones_sb[H : H + 1, :],
                    dn[H : H + 1, :],
                    start=True,
                    stop=True,
                )
                rep = finp.tile([H, QB], F32, tag="rep")
                nc.vector.tensor_copy(rep[:], p_rep[0:H, :])
                osb = finp.tile([H, QB], F32, tag="osb")
                nc.vector.tensor_tensor(osb[:], p_av[0:H, :], rep[:], MUL)
                nc.sync.dma_start(out[:, qs], osb[:])

            # proj runs one chunk ahead of attn so the PE never waits on the
            # psum->sbuf->(kT dma) chain at a chunk boundary.
            proj(0)
            for c in range(1, QC):
                proj(c)
                attn(c - 1)
            attn(QC - 1)

    nc.compile()
    return nc


def _host_inputs(x, Wq, bq, Wk, bk, Wv, bv):
    x = np.asarray(x, np.float32)
    Wq, bq = np.asarray(Wq, np.float32), np.asarray(bq, np.float32)
    Wk, bk = np.asarray(Wk, np.float32), np.asarray(bk, np.float32)
    Wv, bv = np.asarray(Wv, np.float32), np.asarray(bv, np.float32)

    sc = np.float32(1.0 / np.sqrt(H))
    wqk = np.ascontiguousarray(np.concatenate([Wq.T, Wk.T], axis=1))  # [E, 2H]
    wv = np.ascontiguousarray(Wv.T)  # [E, H]
    sb = np.stack(
        [
            np.concatenate([np.full(H, sc, np.float32), np.ones(H, np.float32)]),
            np.concatenate([bq * sc, bk]),
        ],
        axis=1,
    ).astype(np.float32)  # [128, 2]: col0 scale, col1 bias
    bvc = np.ascontiguousarray(bv.reshape(H, 1))
    j = np.arange(QB + 384, dtype=np.int64)[None, :]
    k = np.arange(PB, dtype=np.int64)[:, None]
    mask = (j >= k + 384).astype(np.float32)  # [128, 896] ramp
    ident = np.eye(H, dtype=np.float32)

    shared = {
        "wqk": wqk,
        "wv": wv,
        "sb": sb,
        "bv": bvc,
        "mask": mask,
        "ident": ident,
        "ones": np.ones((PB, H), np.float32),
    }
    in_maps = []
    for b in range(B):
        m = dict(shared)
        m["xT"] = np.ascontiguousarray(x[b].T)
        in_maps.append(m)
    return in_maps


def get_nc():
    if "nc" not in _CACHE:
        _CACHE["nc"] = _build_nc()
    return _CACHE["nc"]


def kernel(x, Wq, bq, Wk, bk, Wv, bv):
    nc = get_nc()
    in_maps = _host_inputs(x, Wq, bq, Wk, bk, Wv, bv)
    results = bass2jax.run_bass_via_pjrt(nc, in_maps, n_cores=NCORES)
    out = np.empty((B, S, H), np.float32)
    for b in range(B):
        out[b] = results[b]["out"].T
    return out


# revision 22
# speedup vs baseline: 1.2256x; 1.2256x over previous
"""Single-head causal attention on 8 Trainium2 NeuronCores.

Problem: x:[8,2048,1024], Wq/Wk/Wv:[64,1024], bq/bk/bv:[64]
  q,k,v = x@W*.T + b*;  out = softmax(causal(q@k.T)/sqrt(64)) @ v

Sharding: batch dim (8) across the 8 cores — fully data-parallel, no
collectives. Each core computes one batch's attention head.

Per-core device kernel (all matmuls fp32r = tf32, full-rate at N=512):
  - host supplies x transposed (xT [E,S]) so E (the contraction dim of the
    projections) lands on SBUF partitions; x streams over two DMA queues
    (sync + gpsimd) in [128,512] tiles.
  - projections: psum_qk[128,512] accumulates [Wq|Wk].T packed (M=128),
    psum_v[64,512] accumulates Wv.T, over 8 e-tiles of 128.
  - q is pre-scaled by 1/sqrt(64) (scale folded into the PSUM->SBUF copy,
    bias folded there too), so scores come out pre-scaled.
  - k must sit at partitions 0:64 for the scores matmul but lands at 64:128
    of the packed projection; it is re-based with a constant permutation
    matmul (cheaper than an SBUF->SBUF DMA competing with the x stream).
  - scores kept transposed: sT[k,q] = kT.T @ qT per (k-tile 128, q-chunk 512).
    Causally-dead tiles are skipped outright; diagonal tiles are masked by
    multiplying exp(s) with a 0/1 ramp mask.
  - softmax without max-subtraction (scores/8 ~ N(0,1); max < ~6, exp safe
    in fp32) -> denominator = column sum of exp(sT), obtained for free as
    row 64 of the attention*V matmul by appending a ones-row to V.
  - V needs k on partitions for the AV matmul: vT tiles are transposed
    128-column-wise on the PE (identity matmul).
  - normalize on device: reciprocal of the denominator row, broadcast
    across the 64 head partitions with a K=1 matmul, multiply.
  - output written h-major ([64, 2048]); host transposes back.
  - emission is interleaved with generators: proj(c) and attn(c-1) alternate
    so the in-order engine queues see attention work during DMA waits, and
    scores run LOOKAHEAD k-tiles ahead of the AV consumer.
"""

import numpy as np

import concourse.bacc as bacc
import concourse.mybir as mybir
import concourse.tile as tile
from concourse import bass2jax

B, S, E, H = 8, 2048, 1024, 64
NCORES = 8
PB = 128  # partition block / k-tile size
QB = 512  # q-chunk (matmul moving free dim)
ET = E // PB  # e-tiles per contraction
QC = S // QB  # q-chunks
KT = S // PB  # k-tiles
DIAG = QB // PB  # diagonal k-tiles per q-chunk
LOOKAHEAD = 3  # scores k-tiles in flight ahead of AV

# packed constants layout: columns of the [128, NCONST] "consts" input
C_SB = 0  # [*, 0:2]   scale/bias (rows 0:128)
C_BV = 2  # [*, 2:3]   v bias (rows 0:64)
C_ID = 3  # [*, 3:67]  identity 64x64 (rows 0:64)
C_ONES = 67  # [*, 67:131] ones
C_PERM = 131  # [*, 131:195] row-rebase permutation (p, h) = 1 iff p == h+64
C_MASK = 195  # [*, 195:1091] causal ramp mask: (p, j) = 1 iff j >= p + 384
NCONST = C_MASK + QB + 384

F32 = mybir.dt.float32
F32R = mybir.dt.float32r
AF = mybir.ActivationFunctionType
MUL = mybir.AluOpType.mult

_CACHE: dict = {}

# schedule/buffering knobs (sweepable)
CFG = {
    "lookahead": 3,
    "xbufs": 12,
    "wtbufs": 5,
    "psbufs": 4,
    "dma2": "gpsimd",  # second x-stream queue
    "wqkv_q": "gpsimd",
    "diag_first": False,
    "attn_first": False,
}


def _interleave(*gens):
    alive = list(gens)
    while alive:
        for g in list(alive):
            try:
                next(g)
            except StopIteration:
                alive.remove(g)


def _build_nc():
    nc = bacc.Bacc("TRN2", target_bir_lowering=False, debug=False)
    xT = nc.dram_tensor("xT", [E, S], F32R, kind="ExternalInput").ap()
    wqkv = nc.dram_tensor("wqkv", [E, 3 * H], F32R, kind="ExternalInput").ap()
    consts = nc.dram_tensor("consts", [PB, NCONST], F32R, kind="ExternalInput").ap()
    out = nc.dram_tensor("out", [H, S], F32, kind="ExternalOutput").ap()

    with tile.TileContext(nc) as tc:
        with (
            tc.tile_pool(name="const", bufs=1) as constp,
            tc.tile_pool(name="xs", bufs=CFG["xbufs"]) as xpool,
            tc.tile_pool(name="qkv", bufs=1) as qkvp,
            tc.tile_pool(name="wt", bufs=CFG["wtbufs"]) as wtp,
            tc.tile_pool(name="fin", bufs=2) as finp,
            tc.tile_pool(name="pqk", bufs=1, space="PSUM") as pqk,
            tc.tile_pool(name="pv", bufs=1, space="PSUM") as pvp,
            tc.tile_pool(name="ps", bufs=CFG["psbufs"], space="PSUM") as psp,
            tc.tile_pool(name="pav", bufs=2, space="PSUM") as pavp,
        ):
            # wqkv gates the first projection matmul: head of the sync queue.
            # cs is not needed until the first PSUM->SBUF copy: gpsimd queue.
            wqkv_sb = constp.tile([PB, ET, 3 * H], F32R)
            _wq = getattr(nc, CFG["wqkv_q"])
            _wq.dma_start(wqkv_sb[:], wqkv.rearrange("(t p) m -> p t m", p=PB))
            cs = constp.tile([PB, NCONST], F32R)
            nc.gpsimd.dma_start(cs[:], consts[:])

            scale_ap = cs[:, C_SB : C_SB + 1].bitcast(F32)
            bias_ap = cs[:, C_SB + 1 : C_SB + 2].bitcast(F32)
            bv_ap = cs[0:H, C_BV : C_BV + 1].bitcast(F32)
            id_ap = cs[0:H, C_ID : C_ID + H].bitcast(F32)
            ones_ap = cs[:, C_ONES : C_ONES + H]
            perm_ap = cs[:, C_PERM : C_PERM + H]

            qkT = qkvp.tile([PB, S], F32R)  # rows 0:64 = q/8, 64:128 = k
            kT = qkvp.tile([H, S], F32R)  # k re-based to partitions 0:64
            vT = qkvp.tile([H, S], F32)  # v h-major (bias applied)
            vsb = qkvp.tile([PB, KT, H + 1], F32R)  # v k-major + ones col
            for m in range(KT):
                nc.vector.tensor_copy(vsb[:, m, H : H + 1], ones_ap[:, 0:1])

            def proj(c):
                qs = slice(c * QB, (c + 1) * QB)
                p_qk = pqk.tile([PB, QB], F32, tag="pqk")
                p_v = pvp.tile([H, QB], F32, tag="pv")
                for e in range(ET):
                    xt = xpool.tile([PB, QB], F32R, tag="xt")
                    dma_eng = nc.sync if e % 2 == 0 else getattr(nc, CFG["dma2"])
                    dma_eng.dma_start(xt[:], xT[e * PB : (e + 1) * PB, qs])
                    nc.tensor.matmul(
                        p_qk[:],
                        wqkv_sb[:, e, 0 : 2 * H],
                        xt[:],
                        start=(e == 0),
                        stop=(e == ET - 1),
                    )
                    nc.tensor.matmul(
                        p_v[:],
                        wqkv_sb[:, e, 2 * H : 3 * H],
                        xt[:],
                        start=(e == 0),
                        stop=(e == ET - 1),
                    )
                    yield
                nc.scalar.activation(
                    qkT[:, qs], p_qk[:], AF.Identity, bias=bias_ap, scale=scale_ap
                )
                nc.scalar.activation(vT[:, qs], p_v[:], AF.Identity, bias=bv_ap)
                yield
                # re-base k rows 64:128 -> 0:64 via permutation matmul
                p_k = pqk.tile([PB, QB], F32, tag="pqk")
                nc.tensor.matmul(p_k[0:H, :], perm_ap, qkT[:, qs], start=True, stop=True)
                nc.vector.tensor_copy(kT[:, qs], p_k[0:H, :])
                yield
                for t in range(DIAG):
                    m = DIAG * c + t
                    p_vt = pvp.tile([PB, H], F32, tag="pv")
                    nc.tensor.transpose(p_vt[:], vT[:, m * PB : (m + 1) * PB], id_ap)
                    nc.vector.tensor_copy(vsb[:, m, 0:H], p_vt[:])
                    if t % 2 == 1:
                        yield

            def attn(c):
                qs = slice(c * QB, (c + 1) * QB)
                nkt = DIAG * c + DIAG
                p_av = pavp.tile([H + 1, QB], F32, tag="pav")

                def weights_tile(m):
                    # scores -> exp -> (diagonal) causal mask
                    p_s = psp.tile([PB, QB], F32, tag="ps")
                    nc.tensor.matmul(
                        p_s[:],
                        kT[:, m * PB : (m + 1) * PB],
                        qkT[0:H, qs],
                        start=True,
                        stop=True,
                    )
                    w = wtp.tile([PB, QB], F32R, tag="w")
                    nc.scalar.activation(w[:], p_s[:], AF.Exp)
                    i = m - DIAG * c
                    if i >= 0:
                        nc.vector.tensor_tensor(
                            w[:],
                            w[:],
                            cs[:, C_MASK + 384 - PB * i : C_MASK + 384 - PB * i + QB],
                            MUL,
                        )
                    return w

                L = CFG["lookahead"]
                if CFG["diag_first"]:
                    order = list(range(DIAG * c, nkt)) + list(range(0, DIAG * c))
                else:
                    order = list(range(nkt))
                ws = {m: weights_tile(m) for m in order[: min(L, nkt)]}
                yield
                for idx, m in enumerate(order):
                    if idx + L < nkt:
                        ws[order[idx + L]] = weights_tile(order[idx + L])
                    nc.tensor.matmul(
                        p_av[:],
                        vsb[:, m, :],
                        ws.pop(m),
                        start=(idx == 0),
                        stop=(idx == nkt - 1),
                    )
                    yield
                # normalize: out[h, q] = av[h, q] / av[64, q]
                dn = wtp.tile([PB, QB], F32R, tag="w")
                with nc.allow_low_precision("fp32r denominators feed an fp32r matmul"):
                    nc.vector.reciprocal(dn[H : H + 1, :], p_av[H : H + 1, :])
                p_rep = pavp.tile([H + 1, QB], F32, tag="pav")
                nc.tensor.matmul(
                    p_rep[0:H, :],
                    ones_ap[H : H + 1, :],
                    dn[H : H + 1, :],
                    start=True,
                    stop=True,
                )
                yield
                rep = finp.tile([H, QB], F32, tag="rep")
                nc.vector.tensor_copy(rep[:], p_rep[0:H, :])
                osb = finp.tile([H, QB], F32, tag="osb")
                nc.vector.tensor_tensor(osb[:], p_av[0:H, :], rep[:], MUL)
                nc.sync.dma_start(out[:, qs], osb[:])
                yield

            # interleaved emission: proj(c) alternates with attn(c-1) so the
            # in-order engine queues see attention work during DMA waits.
            _interleave(proj(0))
            for c in range(1, QC):
                if CFG["attn_first"]:
                    _interleave(attn(c - 1), proj(c))
                else:
                    _interleave(proj(c), attn(c - 1))
            _interleave(attn(QC - 1))

    nc.compile()
    return nc


def _host_inputs(x, Wq, bq, Wk, bk, Wv, bv):
    x = np.asarray(x, np.float32)
    Wq, bq = np.asarray(Wq, np.float32), np.asarray(bq, np.float32)
    Wk, bk = np.asarray(Wk, np.float32), np.asarray(bk, np.float32)
    Wv, bv = np.asarray(Wv, np.float32), np.asarray(bv, np.float32)

    sc = np.float32(1.0 / np.sqrt(H))
    wqkv = np.ascontiguousarray(np.concatenate([Wq.T, Wk.T, Wv.T], axis=1))  # [E, 3H]

    cs = np.zeros((PB, NCONST), np.float32)
    cs[:, C_SB] = np.concatenate([np.full(H, sc, np.float32), np.ones(H, np.float32)])
    cs[:, C_SB + 1] = np.concatenate([bq * sc, bk])
    cs[:H, C_BV] = bv
    cs[:H, C_ID : C_ID + H] = np.eye(H, dtype=np.float32)
    cs[:, C_ONES : C_ONES + H] = 1.0
    cs[H:PB, C_PERM : C_PERM + H] = np.eye(H, dtype=np.float32)
    j = np.arange(QB + 384, dtype=np.int64)[None, :]
    k = np.arange(PB, dtype=np.int64)[:, None]
    cs[:, C_MASK:] = (j >= k + 384).astype(np.float32)

    shared = {"wqkv": wqkv, "consts": cs}
    in_maps = []
    for b in range(B):
        m = dict(shared)
        m["xT"] = np.ascontiguousarray(x[b].T)
        in_maps.append(m)
    return in_maps


def get_nc():
    if "nc" not in _CACHE:
        _CACHE["nc"] = _build_nc()
    return _CACHE["nc"]


def kernel(x, Wq, bq, Wk, bk, Wv, bv):
    nc = get_nc()
    in_maps = _host_inputs(x, Wq, bq, Wk, bk, Wv, bv)
    results = bass2jax.run_bass_via_pjrt(nc, in_maps, n_cores=NCORES)
    out = np.empty((B, S, H), np.float32)
    for b in range(B):
        out[b] = results[b]["out"].T
    return out


# revision 24
# speedup vs baseline: 1.2590x; 1.0272x over previous
"""Single-head causal attention on 8 Trainium2 NeuronCores.

Problem: x:[8,2048,1024], Wq/Wk/Wv:[64,1024], bq/bk/bv:[64]
  q,k,v = x@W*.T + b*;  out = softmax(causal(q@k.T)/sqrt(64)) @ v

Sharding: batch dim (8) across the 8 cores — fully data-parallel, no
collectives. Each core computes one batch's attention head.

Per-core device kernel (all matmuls fp32r = tf32, full-rate at N=512):
  - host supplies x transposed (xT [E,S]) so E (the contraction dim of the
    projections) lands on SBUF partitions; x streams over two DMA queues
    (sync + gpsimd) in [128,512] tiles.
  - projections: psum_qk[128,512] accumulates [Wq|Wk].T packed (M=128),
    psum_v[64,512] accumulates Wv.T, over 8 e-tiles of 128.
  - q is pre-scaled by 1/sqrt(64) (scale folded into the PSUM->SBUF copy,
    bias folded there too), so scores come out pre-scaled.
  - k must sit at partitions 0:64 for the scores matmul but lands at 64:128
    of the packed projection; it is re-based with a constant permutation
    matmul (cheaper than an SBUF->SBUF DMA competing with the x stream).
  - scores kept transposed: sT[k,q] = kT.T @ qT per (k-tile 128, q-chunk 512).
    Causally-dead tiles are skipped outright; diagonal tiles are masked by
    multiplying exp(s) with a 0/1 ramp mask.
  - softmax without max-subtraction (scores/8 ~ N(0,1); max < ~6, exp safe
    in fp32) -> denominator = column sum of exp(sT), obtained for free as
    row 64 of the attention*V matmul by appending a ones-row to V.
  - V needs k on partitions for the AV matmul: vT tiles are transposed
    128-column-wise on the PE (identity matmul).
  - normalize on device: reciprocal of the denominator row, broadcast
    across the 64 head partitions with a K=1 matmul, multiply.
  - output written h-major ([64, 2048]); host transposes back.
  - emission is interleaved with generators: proj(c) and attn(c-1) alternate
    so the in-order engine queues see attention work during DMA waits, and
    scores run LOOKAHEAD k-tiles ahead of the AV consumer.
"""

import numpy as np

import concourse.bacc as bacc
import concourse.mybir as mybir
import concourse.tile as tile
from concourse import bass2jax

B, S, E, H = 8, 2048, 1024, 64
NCORES = 8
PB = 128  # partition block / k-tile size
QB = 512  # q-chunk (matmul moving free dim)
ET = E // PB  # e-tiles per contraction
QC = S // QB  # q-chunks
KT = S // PB  # k-tiles
DIAG = QB // PB  # diagonal k-tiles per q-chunk
LOOKAHEAD = 3  # scores k-tiles in flight ahead of AV

# packed constants layout: columns of the [128, NCONST] "consts" input
C_SB = 0  # [*, 0:2]   scale/bias (rows 0:128)
C_BV = 2  # [*, 2:3]   v bias (rows 0:64)
C_ID = 3  # [*, 3:67]  identity 64x64 (rows 0:64)
C_ONES = 67  # [*, 67:131] ones
C_PERM = 131  # [*, 131:195] row-rebase permutation (p, h) = 1 iff p == h+64
C_MASK = 195  # [*, 195:1091] causal ramp mask: (p, j) = 1 iff j >= p + 384
NCONST = C_MASK + QB + 384

F32 = mybir.dt.float32
F32R = mybir.dt.float32r
AF = mybir.ActivationFunctionType
MUL = mybir.AluOpType.mult

_CACHE: dict = {}

# schedule/buffering knobs (sweepable)
CFG = {
    "lookahead": 5,
    "xbufs": 12,
    "wtbufs": 8,
    "psbufs": 4,
    "dma2": "gpsimd",  # second x-stream queue
    "wqkv_q": "gpsimd",
    "diag_first": False,
    "attn_first": False,
}


def _interleave(*gens):
    """Drive generators round-robin; the first (proj) gets two steps per turn."""
    alive = list(gens)
    steps = {id(g): (2 if i == 0 and len(gens) > 1 else 1) for i, g in enumerate(gens)}
    while alive:
        for g in list(alive):
            for _ in range(steps[id(g)]):
                try:
                    next(g)
                except StopIteration:
                    alive.remove(g)
                    break


def _build_nc():
    nc = bacc.Bacc("TRN2", target_bir_lowering=False, debug=False)
    xT = nc.dram_tensor("xT", [E, S], F32R, kind="ExternalInput").ap()
    wqkv = nc.dram_tensor("wqkv", [E, 3 * H], F32R, kind="ExternalInput").ap()
    consts = nc.dram_tensor("consts", [PB, NCONST], F32R, kind="ExternalInput").ap()
    out = nc.dram_tensor("out", [H, S], F32, kind="ExternalOutput").ap()

    with tile.TileContext(nc) as tc:
        with (
            tc.tile_pool(name="const", bufs=1) as constp,
            tc.tile_pool(name="xs", bufs=CFG["xbufs"]) as xpool,
            tc.tile_pool(name="qkv", bufs=1) as qkvp,
            tc.tile_pool(name="wt", bufs=CFG["wtbufs"]) as wtp,
            tc.tile_pool(name="fin", bufs=2) as finp,
            tc.tile_pool(name="pqk", bufs=1, space="PSUM") as pqk,
            tc.tile_pool(name="pv", bufs=1, space="PSUM") as pvp,
            tc.tile_pool(name="ps", bufs=CFG["psbufs"], space="PSUM") as psp,
            tc.tile_pool(name="pav", bufs=2, space="PSUM") as pavp,
        ):
            # wqkv gates the first projection matmul: head of the sync queue.
            # cs is not needed until the first PSUM->SBUF copy: gpsimd queue.
            wqkv_sb = constp.tile([PB, ET, 3 * H], F32R)
            _wq = getattr(nc, CFG["wqkv_q"])
            _wq.dma_start(wqkv_sb[:], wqkv.rearrange("(t p) m -> p t m", p=PB))
            cs = constp.tile([PB, NCONST], F32R)
            nc.gpsimd.dma_start(cs[:], consts[:])

            scale_ap = cs[:, C_SB : C_SB + 1].bitcast(F32)
            bias_ap = cs[:, C_SB + 1 : C_SB + 2].bitcast(F32)
            bv_ap = cs[0:H, C_BV : C_BV + 1].bitcast(F32)
            id_ap = cs[0:H, C_ID : C_ID + H].bitcast(F32)
            ones_ap = cs[:, C_ONES : C_ONES + H]
            perm_ap = cs[:, C_PERM : C_PERM + H]

            qkT = qkvp.tile([PB, S], F32R)  # rows 0:64 = q/8, 64:128 = k
            kT = qkvp.tile([H, S], F32R)  # k re-based to partitions 0:64
            vT = qkvp.tile([H, S], F32)  # v h-major (bias applied)
            vsb = qkvp.tile([PB, KT, H + 1], F32R)  # v k-major + ones col
            for m in range(KT):
                nc.vector.tensor_copy(vsb[:, m, H : H + 1], ones_ap[:, 0:1])

            def proj(c):
                qs = slice(c * QB, (c + 1) * QB)
                p_qk = pqk.tile([PB, QB], F32, tag="pqk")
                p_v = pvp.tile([H, QB], F32, tag="pv")
                for e in range(ET):
                    xt = xpool.tile([PB, QB], F32R, tag="xt")
                    dma_eng = nc.sync if e % 2 == 0 else getattr(nc, CFG["dma2"])
                    dma_eng.dma_start(xt[:], xT[e * PB : (e + 1) * PB, qs])
                    nc.tensor.matmul(
                        p_qk[:],
                        wqkv_sb[:, e, 0 : 2 * H],
                        xt[:],
                        start=(e == 0),
                        stop=(e == ET - 1),
                    )
                    nc.tensor.matmul(
                        p_v[:],
                        wqkv_sb[:, e, 2 * H : 3 * H],
                        xt[:],
                        start=(e == 0),
                        stop=(e == ET - 1),
                    )
                    yield
                nc.scalar.activation(
                    qkT[:, qs], p_qk[:], AF.Identity, bias=bias_ap, scale=scale_ap
                )
                nc.scalar.activation(vT[:, qs], p_v[:], AF.Identity, bias=bv_ap)
                yield
                # re-base k rows 64:128 -> 0:64 via permutation matmul
                p_k = pqk.tile([PB, QB], F32, tag="pqk")
                nc.tensor.matmul(p_k[0:H, :], perm_ap, qkT[:, qs], start=True, stop=True)
                nc.vector.tensor_copy(kT[:, qs], p_k[0:H, :])
                yield
                for t in range(DIAG):
                    m = DIAG * c + t
                    p_vt = pvp.tile([PB, H], F32, tag="pv")
                    nc.tensor.transpose(p_vt[:], vT[:, m * PB : (m + 1) * PB], id_ap)
                    nc.vector.tensor_copy(vsb[:, m, 0:H], p_vt[:])
                    if t % 2 == 1:
                        yield

            def attn(c):
                qs = slice(c * QB, (c + 1) * QB)
                nkt = DIAG * c + DIAG
                p_av = pavp.tile([H + 1, QB], F32, tag="pav")

                def weights_tile(m):
                    # scores -> exp -> (diagonal) causal mask
                    p_s = psp.tile([PB, QB], F32, tag="ps")
                    nc.tensor.matmul(
                        p_s[:],
                        kT[:, m * PB : (m + 1) * PB],
                        qkT[0:H, qs],
                        start=True,
                        stop=True,
                    )
                    w = wtp.tile([PB, QB], F32R, tag="w")
                    nc.scalar.activation(w[:], p_s[:], AF.Exp)
                    i = m - DIAG * c
                    if i >= 0:
                        nc.vector.tensor_tensor(
                            w[:],
                            w[:],
                            cs[:, C_MASK + 384 - PB * i : C_MASK + 384 - PB * i + QB],
                            MUL,
                        )
                    return w

                L = CFG["lookahead"]
                if CFG["diag_first"]:
                    order = list(range(DIAG * c, nkt)) + list(range(0, DIAG * c))
                else:
                    order = list(range(nkt))
                ws = {m: weights_tile(m) for m in order[: min(L, nkt)]}
                yield
                for idx, m in enumerate(order):
                    if idx + L < nkt:
                        ws[order[idx + L]] = weights_tile(order[idx + L])
                    nc.tensor.matmul(
                        p_av[:],
                        vsb[:, m, :],
                        ws.pop(m),
                        start=(idx == 0),
                        stop=(idx == nkt - 1),
                    )
                    yield
                # normalize: out[h, q] = av[h, q] / av[64, q]
                dn = wtp.tile([PB, QB], F32R, tag="w")
                with nc.allow_low_precision("fp32r denominators feed an fp32r matmul"):
                    nc.vector.reciprocal(dn[H : H + 1, :], p_av[H : H + 1, :])
                p_rep = pavp.tile([H + 1, QB], F32, tag="pav")
                nc.tensor.matmul(
                    p_rep[0:H, :],
                    ones_ap[H : H + 1, :],
                    dn[H : H + 1, :],
                    start=True,
                    stop=True,
                )
                yield
                rep = finp.tile([H, QB], F32, tag="rep")
                nc.vector.tensor_copy(rep[:], p_rep[0:H, :])
                osb = finp.tile([H, QB], F32, tag="osb")
                nc.vector.tensor_tensor(osb[:], p_av[0:H, :], rep[:], MUL)
                nc.sync.dma_start(out[:, qs], osb[:])
                yield

            # interleaved emission: proj(c) alternates with attn(c-1) so the
            # in-order engine queues see attention work during DMA waits.
            _interleave(proj(0))
            for c in range(1, QC):
                if CFG["attn_first"]:
                    _interleave(attn(c - 1), proj(c))
                else:
                    _interleave(proj(c), attn(c - 1))
            _interleave(attn(QC - 1))

    nc.compile()
    return nc


def _host_inputs(x, Wq, bq, Wk, bk, Wv, bv):
    x = np.asarray(x, np.float32)
    Wq, bq = np.asarray(Wq, np.float32), np.asarray(bq, np.float32)
    Wk, bk = np.asarray(Wk, np.float32), np.asarray(bk, np.float32)
    Wv, bv = np.asarray(Wv, np.float32), np.asarray(bv, np.float32)

    sc = np.float32(1.0 / np.sqrt(H))
    wqkv = np.ascontiguousarray(np.concatenate([Wq.T, Wk.T, Wv.T], axis=1))  # [E, 3H]

    cs = np.zeros((PB, NCONST), np.float32)
    cs[:, C_SB] = np.concatenate([np.full(H, sc, np.float32), np.ones(H, np.float32)])
    cs[:, C_SB + 1] = np.concatenate([bq * sc, bk])
    cs[:H, C_BV] = bv
    cs[:H, C_ID : C_ID + H] = np.eye(H, dtype=np.float32)
    cs[:, C_ONES : C_ONES + H] = 1.0
    cs[H:PB, C_PERM : C_PERM + H] = np.eye(H, dtype=np.float32)
    j = np.arange(QB + 384, dtype=np.int64)[None, :]
    k = np.arange(PB, dtype=np.int64)[:, None]
    cs[:, C_MASK:] = (j >= k + 384).astype(np.float32)

    shared = {"wqkv": wqkv, "consts": cs}
    in_maps = []
    for b in range(B):
        m = dict(shared)
        m["xT"] = np.ascontiguousarray(x[b].T)
        in_maps.append(m)
    return in_maps


def get_nc():
    if "nc" not in _CACHE:
        _CACHE["nc"] = _build_nc()
    return _CACHE["nc"]


def kernel(x, Wq, bq, Wk, bk, Wv, bv):
    nc = get_nc()
    in_maps = _host_inputs(x, Wq, bq, Wk, bk, Wv, bv)
    results = bass2jax.run_bass_via_pjrt(nc, in_maps, n_cores=NCORES)
    out = np.empty((B, S, H), np.float32)
    for b in range(B):
        out[b] = results[b]["out"].T
    return out
